# revision 36
# baseline (speedup 1.0000x reference)
"""Trainium2 Bass kernel for a dense transformer encoder layer (v3).

Reference computation (per batch b):
    q = x.reshape(L, H, E)                       # H=16 heads, E=64
    scores = q @ q^T per head, scaled softmax    # A = softmax(s/8)
    new_x  = concat_h(A_h @ q_h)                 # [L, D]
    x1 = LN(x + new_x; g1, be1)
    y  = relu(x1 @ w1^T + b1) @ w2^T + b2
    out = LN(x1 + y; g2, be2)

Sharding: pure data parallel over (batch, seq-half): core c handles
batch c//2, query rows [(c%2)*1024, +1024).  Keys/values span the full
sequence of that batch, so every core gets the whole x[b] (queries
reordered first) and the full FFN weights.  No device collectives.

Structure (per core, all matmuls bf16):
  - x^T built by DMA-engine XBAR transposes; V (with a ones column per
    head for the softmax denominators) is DMA'd pre-interleaved from
    the host.
  - attention pipelined at u-tile granularity: scores(u) -> exp(u) on
    the ACT engine -> AV(u) lagging LAG u-steps so the PE never waits
    on exp.  The per-head U^T epilogue (PE transpose + divide) is
    emitted one head late as PE gap filler.
  - LN1/LN2 run in two phases over all row tiles (stats+sqrt first,
    then normalize) so the DVE->ACT->DVE latency is paid once, not per
    row tile.  be1 is folded into the FFN biases host-side.
  - FFN weights stream as per-block-contiguous [P, n*P] DMAs on the
    hardware DGE queues; each stationary serves both 512-col slabs;
    FFN2 accumulates all 32 f-tiles in PSUM.
"""

import numpy as np

import concourse.bass as bass
import concourse.tile as tile
from concourse import bacc
from concourse import mybir
from concourse.masks import make_identity

F32 = mybir.dt.float32
BF16 = mybir.dt.bfloat16
I16 = mybir.dt.int16
EXP = mybir.ActivationFunctionType.Exp

# Schraudolph-style exp in the bf16 bit domain: the bf16 bit pattern of
# exp(s/8) = 2^(s*log2(e)/8) is approximately int16(s*SCHRA_A + SCHRA_B).
# Max rel err ~3.3%; used only for off-diagonal score tiles, where no
# single softmax weight is large, so the error is ~1e-3 on the output.
SCHRA_A = float(np.log2(np.e) / 8.0 * 128.0)
SCHRA_B = 16256.0 - 5.5
RELU = mybir.ActivationFunctionType.Relu
SQRT = mybir.ActivationFunctionType.Sqrt
SUB = mybir.AluOpType.subtract
MUL = mybir.AluOpType.mult

LN_EPS = 1e-5
E = 64          # head dim
W = E + 1       # head dim + ones column
P = 128         # partitions


def build_program(S=2048, D=1024, F=4096, n_cores_unused=8):
    H = D // E
    Lq = S // 2
    ST = S // P          # s-tiles
    LT = Lq // P         # query row tiles
    DT = D // P          # d chunks
    FT = F // P          # f tiles
    SL = 512             # moving slab width
    NSL = Lq // SL
    GS = min(512, D)     # bn_stats subgroup size
    LAG = 3              # AV lags scores by LAG u-steps

    nc = bacc.Bacc("TRN2")

    xb = nc.dram_tensor("xb", [Lq, D], F32, kind="ExternalInput")
    xb16 = nc.dram_tensor("xb16", [S, D], BF16, kind="ExternalInput")
    xbv = nc.dram_tensor("xbv", [S, H, W], BF16, kind="ExternalInput")
    w1th = nc.dram_tensor("w1th", [FT, P, DT, P], BF16, kind="ExternalInput")
    w2th = nc.dram_tensor("w2th", [DT, P, FT, P], BF16, kind="ExternalInput")
    b1 = nc.dram_tensor("b1", [F], F32, kind="ExternalInput")
    b2 = nc.dram_tensor("b2", [D], F32, kind="ExternalInput")
    g1 = nc.dram_tensor("g1", [D], F32, kind="ExternalInput")
    g2 = nc.dram_tensor("g2", [D], F32, kind="ExternalInput")
    be2 = nc.dram_tensor("be2", [D], F32, kind="ExternalInput")
    out = nc.dram_tensor("out", [Lq, D], F32, kind="ExternalOutput")

    def bcast(dram_vec, n):
        a = dram_vec[:]
        return bass.AP(tensor=a.tensor, offset=a.offset, ap=[[0, P]] + a.ap)

    with tile.TileContext(nc) as tc:
        with (
            tc.tile_pool(name="persist", bufs=1) as persist,
            tc.tile_pool(name="small", bufs=6) as small,
            tc.tile_pool(name="w1p", bufs=4) as w1p,
        ):
            ident = persist.tile([P, P], F32)
            make_identity(nc, ident)
            ident16 = persist.tile([P, P], BF16)
            make_identity(nc, ident16)
            b1s = persist.tile([P, FT], F32)
            nc.sync.dma_start(out=b1s, in_=b1[:].rearrange("(t p) -> p t", p=P))
            b2s = persist.tile([P, DT], F32)
            nc.sync.dma_start(out=b2s, in_=b2[:].rearrange("(t p) -> p t", p=P))
            epst = persist.tile([P, 1], F32)
            nc.vector.memset(epst, LN_EPS)
            g1b = persist.tile([P, D], F32)
            nc.gpsimd.dma_start(out=g1b, in_=bcast(g1, D))
            g2b = persist.tile([P, D], F32)
            nc.gpsimd.dma_start(out=g2b, in_=bcast(g2, D))
            be2b = persist.tile([P, D], F32)
            nc.gpsimd.dma_start(out=be2b, in_=bcast(be2, D))
            # new_x: attention output, then r1/LN1 scratch, then r2.
            new_x = persist.tile([P, LT, D], F32)
            x1b = persist.tile([P, LT, D], BF16)   # LN1 out * g1 (no be1)
            x1T = persist.tile([P, DT, Lq], BF16)  # x1b transposed
            mvall = persist.tile([P, LT, 2], F32)  # bn mean/var per row tile
            rsall = persist.tile([P, LT, 1], F32)  # rstd per row tile

            # ---------------- attention ----------------
            with (
                tc.tile_pool(name="asb", bufs=1) as asb,
                tc.tile_pool(name="etp", bufs=5) as etp,
                tc.tile_pool(name="utsp", bufs=2) as utsp,
                tc.tile_pool(name="xqp", bufs=8) as xqp,
                tc.tile_pool(name="scp", bufs=2, space="PSUM") as scp,
                tc.tile_pool(name="utp", bufs=2, space="PSUM") as utp,
                tc.tile_pool(name="upp", bufs=2, space="PSUM") as upp,
            ):
                vaug = asb.tile([P, ST, H, W], BF16)
                xT = asb.tile([P, DT, S], BF16)

                def emit_xt(t):
                    nc.sync.dma_start_transpose(
                        out=xT[:, t, :], in_=xb16[:, t * P:(t + 1) * P])

                def emit_vaug(u):
                    nc.sync.dma_start(
                        out=vaug[:, u, :, :],
                        in_=xbv[u * P:(u + 1) * P, :, :])

                # head 0 needs xT col 0 + the first vaug tiles first.
                emit_xt(0)
                for u in range(4):
                    emit_vaug(u)
                emit_xt(1)
                for u in range(4, ST):
                    emit_vaug(u)
                for t in range(2, DT):
                    emit_xt(t)

                uts_saved = {}

                def emit_epilogue(h):
                    # transpose U^T [65, Lq] back to [l, 65] in 128-row
                    # tiles, divide by the rowsum riding in column 64.
                    uts = uts_saved.pop(h)
                    for half in range(LT // 4):
                        up = upp.tile([P, 4, W], F32, tag="up")
                        for j in range(4):
                            lt = half * 4 + j
                            nc.tensor.transpose(
                                up[:, j, :], uts[:, lt * P:(lt + 1) * P],
                                ident[0:W, 0:W])
                        nc.vector.reciprocal(
                            out=up[:, :, E:W], in_=up[:, :, E:W])
                        for j in range(4):
                            lt = half * 4 + j
                            nc.vector.tensor_scalar_mul(
                                out=new_x[:, lt, h * E:(h + 1) * E],
                                in0=up[:, j, 0:E], scalar1=up[:, j, E:W])

                for h in range(H):
                    t, ro = h // 2, (h % 2) * E
                    ut_sl = [utp.tile([W, SL], F32, tag="ut", name="ut")
                             for _ in range(NSL)]
                    ets = {}

                    def emit_scores(u):
                        sc = scp.tile([P, Lq], F32, tag="sc")
                        for sl in range(NSL):
                            nc.tensor.matmul(
                                sc[:, sl * SL:(sl + 1) * SL],
                                xT[ro:ro + E, t, u * P:(u + 1) * P],
                                xT[ro:ro + E, t, sl * SL:(sl + 1) * SL],
                                start=True, stop=True)
                        et = etp.tile([P, Lq], BF16, tag="et")
                        if u < ST // 2:
                            # s-tiles containing the diagonal: exact exp
                            # on the ACT engine.
                            nc.scalar.activation(
                                out=et, in_=sc, func=EXP, scale=1.0 / 8.0)
                        else:
                            # off-diagonal s-tiles: Schraudolph exp on the
                            # (otherwise idle) DVE to relieve ACT.
                            nc.vector.tensor_scalar(
                                out=et.bitcast(I16), in0=sc,
                                scalar1=SCHRA_A, scalar2=SCHRA_B,
                                op0=MUL, op1=mybir.AluOpType.add)
                        ets[u] = et

                    def emit_av(u):
                        et = ets.pop(u)
                        for sl in range(NSL):
                            nc.tensor.matmul(
                                ut_sl[sl], vaug[:, u, h, 0:W],
                                et[:, sl * SL:(sl + 1) * SL],
                                start=(u == 0), stop=(u == ST - 1))

                    for u in range(ST):
                        emit_scores(u)
                        if u == 2 and h > 0:
                            emit_epilogue(h - 1)
                        if u >= LAG:
                            emit_av(u - LAG)
                    for u in range(ST - LAG, ST):
                        emit_av(u)

                    uts = utsp.tile([W, Lq], F32, tag="uts")
                    for sl in range(NSL):
                        nc.vector.tensor_copy(
                            out=uts[:, sl * SL:(sl + 1) * SL], in_=ut_sl[sl])
                    uts_saved[h] = uts
                emit_epilogue(H - 1)

                # residual 1 + LN1 (be1 folded into the FFN biases).
                # Two-phase over row tiles so the DVE->ACT->DVE hop for
                # rsqrt is paid in parallel across tiles.
                for lt in range(LT):
                    xq = xqp.tile([P, D], F32, tag="xq")
                    nc.sync.dma_start(
                        out=xq, in_=xb[lt * P:(lt + 1) * P, :])
                    nc.gpsimd.tensor_add(
                        out=new_x[:, lt, :], in0=new_x[:, lt, :], in1=xq)
                    _ln_stats(nc, small, new_x[:, lt, :],
                              mvall[:, lt, :], rsall[:, lt, :], epst, GS)
                for lt in range(LT):
                    nc.vector.reciprocal(
                        out=rsall[:, lt, :], in_=rsall[:, lt, :])
                    nc.vector.tensor_scalar(
                        out=new_x[:, lt, :], in0=new_x[:, lt, :],
                        scalar1=mvall[:, lt, 0:1], scalar2=rsall[:, lt, :],
                        op0=SUB, op1=MUL)
                    nc.vector.tensor_mul(
                        out=x1b[:, lt, :], in0=new_x[:, lt, :], in1=g1b)

            # ---------------- x1 transpose + FFN (fp8 DoubleRow) -------
            with (
                tc.tile_pool(name="fsb", bufs=1) as fsb,
                tc.tile_pool(name="w2p", bufs=2) as w2p,
                tc.tile_pool(name="ybp", bufs=2) as ybp,
                tc.tile_pool(name="outp", bufs=3) as outp,
            ):
                with tc.tile_pool(name="x1tp", bufs=4, space="PSUM") as x1tp:
                    for lt in range(LT):
                        for dc in range(DT):
                            tp = x1tp.tile([P, P], BF16)
                            nc.tensor.transpose(
                                tp, x1b[:, lt, dc * P:(dc + 1) * P], ident16)
                            nc.vector.tensor_copy(
                                out=x1T[:, dc, lt * P:(lt + 1) * P], in_=tp)

                htall = fsb.tile([P, FT, Lq], BF16)

                with tc.tile_pool(name="hpp", bufs=4, space="PSUM") as hpp:
                    for ft in range(FT):
                        wblk = w1p.tile([P, DT, P], BF16, tag="w1")
                        nc.sync.dma_start(out=wblk, in_=w1th[ft, :, :, :])
                        hp = [hpp.tile([P, SL], F32, tag="hp", name="hp")
                              for _ in range(NSL)]
                        for dc in range(DT):
                            stat = wblk[:, dc, :]
                            for sl in range(NSL):
                                nc.tensor.matmul(
                                    hp[sl], stat,
                                    x1T[:, dc, sl * SL:(sl + 1) * SL],
                                    start=(dc == 0), stop=(dc == DT - 1))
                        for sl in range(NSL):
                            nc.scalar.activation(
                                out=htall[:, ft, sl * SL:(sl + 1) * SL],
                                in_=hp[sl], func=RELU, bias=b1s[:, ft:ft + 1])

                with (
                    tc.tile_pool(name="ypp", bufs=4, space="PSUM") as ypp,
                    tc.tile_pool(name="ytp", bufs=2, space="PSUM") as ytp,
                ):
                    for dt in range(DT):
                        w2blk = w2p.tile([P, FT, P], BF16, tag="w2")
                        nc.scalar.dma_start(out=w2blk, in_=w2th[dt, :, :, :])
                        yp = [ypp.tile([P, SL], F32, tag="yp", name="yp")
                              for _ in range(NSL)]
                        for j in range(FT):
                            stat = w2blk[:, j, :]
                            for sl in range(NSL):
                                nc.tensor.matmul(
                                    yp[sl], stat,
                                    htall[:, j, sl * SL:(sl + 1) * SL],
                                    start=(j == 0), stop=(j == FT - 1))
                        for sl in range(NSL):
                            ybuf = ybp.tile([P, SL], F32, tag="yb")
                            nc.vector.tensor_scalar_add(
                                out=ybuf, in0=yp[sl],
                                scalar1=b2s[:, dt:dt + 1])
                            for c in range(SL // P):
                                lt = sl * (SL // P) + c
                                yt = ytp.tile([P, P], F32)
                                nc.tensor.transpose(
                                    yt, ybuf[:, c * P:(c + 1) * P], ident)
                                nc.vector.tensor_add(
                                    out=new_x[:, lt, dt * P:(dt + 1) * P],
                                    in0=x1b[:, lt, dt * P:(dt + 1) * P],
                                    in1=yt)

                    # LN2 -> out (two-phase again)
                    for lt in range(LT):
                        _ln_stats(nc, small, new_x[:, lt, :],
                                  mvall[:, lt, :], rsall[:, lt, :], epst, GS)
                    for lt in range(LT):
                        nc.vector.reciprocal(
                            out=rsall[:, lt, :], in_=rsall[:, lt, :])
                        nc.vector.tensor_scalar(
                            out=new_x[:, lt, :], in0=new_x[:, lt, :],
                            scalar1=mvall[:, lt, 0:1], scalar2=rsall[:, lt, :],
                            op0=SUB, op1=MUL)
                        ot = outp.tile([P, D], F32, tag="ot")
                        nc.vector.tensor_mul(
                            out=ot, in0=new_x[:, lt, :], in1=g2b)
                        nc.vector.tensor_add(out=ot, in0=ot, in1=be2b)
                        nc.sync.dma_start(
                            out=out[lt * P:(lt + 1) * P, :], in_=ot)

    nc.finalize()
    return nc


def _ln_stats(nc, small, x_ap, mv, rstd, epst, GS):
    """mv <- [mean, var] of x over free dim; rstd <- sqrt(var + eps)."""
    D = x_ap.shape[-1]
    ngr = D // GS
    st = small.tile([P, ngr, 6], F32, tag="bnst")
    xg = x_ap.rearrange("p (g k) -> p g k", k=GS)
    for g in range(ngr):
        nc.vector.bn_stats(out=st[:, g, :], in_=xg[:, g, :])
    nc.vector.bn_aggr(out=mv, in_=st)
    nc.scalar.activation(out=rstd, in_=mv[:, 1:2], func=SQRT, bias=epst)


# ---------------------------------------------------------------------------
# host side
# ---------------------------------------------------------------------------

_PROG_CACHE = {}


def get_program(S=2048, D=1024, F=4096):
    key = (S, D, F)
    if key not in _PROG_CACHE:
        _PROG_CACHE[key] = build_program(S, D, F)
    return _PROG_CACHE[key]


def make_in_maps(x, w1, b1, w2, b2, g1, be1, g2, be2, n_cores=8):
    B, L, D = x.shape
    F = w1.shape[0]
    Lq = L // 2
    H, Wd = D // 64, 65
    DT, FT = D // 128, F // 128
    import ml_dtypes
    # per-block-contiguous weight layouts:
    # w1th[ft, p, dc, q] = w1[ft*128+q, dc*128+p]
    w1th = np.ascontiguousarray(
        w1.T.reshape(DT, 128, FT, 128).transpose(2, 1, 0, 3)).astype(ml_dtypes.bfloat16)
    # w2th[dt, p, j, q] = w2[dt*128+q, j*128+p]
    w2th = np.ascontiguousarray(
        w2.T.reshape(FT, 128, DT, 128).transpose(2, 1, 0, 3)).astype(ml_dtypes.bfloat16)
    # be1 folded into the FFN biases: relu(w1 @ (x1 + be1) + b1) =
    # relu(w1 @ x1 + b1'), and r2 = x1 + y + be1 = x1 + (w2 h + b2').
    b1f = (b1 + w1 @ be1).astype(np.float32)
    b2f = (b2 + be1).astype(np.float32)
    common = dict(w1th=w1th, w2th=w2th, b1=b1f, b2=b2f, g1=g1, g2=g2, be2=be2)
    in_maps = []
    for c in range(n_cores):
        b, half = c // 2, c % 2
        lo = half * Lq
        xq = x[b, lo:lo + Lq]
        xo = x[b, Lq - lo:2 * Lq - lo]
        xbl = np.ascontiguousarray(np.concatenate([xq, xo], axis=0))
        # V with a ones column interleaved per head: [S, H, 65]
        xv = np.concatenate(
            [xbl.reshape(L, H, 64), np.ones((L, H, 1), np.float32)],
            axis=2)
        in_maps.append(dict(xb=np.ascontiguousarray(xq),
                            xb16=xbl.astype(ml_dtypes.bfloat16),
                            xbv=np.ascontiguousarray(xv).astype(ml_dtypes.bfloat16),
                            **common))
    return in_maps


def kernel(x, w1, b1, w2, b2, g1, be1, g2, be2):
    from concourse.bass_utils import run_bass_kernel_spmd

    x = np.asarray(x, dtype=np.float32)
    B, L, D = x.shape
    F = w1.shape[0]
    Lq = L // 2
    n_cores = 2 * B
    nc = get_program(L, D, F)
    in_maps = make_in_maps(x, np.asarray(w1, np.float32), np.asarray(b1, np.float32),
                           np.asarray(w2, np.float32), np.asarray(b2, np.float32),
                           np.asarray(g1, np.float32), np.asarray(be1, np.float32),
                           np.asarray(g2, np.float32), np.asarray(be2, np.float32),
                           n_cores)
    res = run_bass_kernel_spmd(nc, in_maps, core_ids=list(range(n_cores)))
    outp = np.empty((B, L, D), dtype=np.float32)
    for c in range(n_cores):
        b, half = c // 2, c % 2
        outp[b, half * Lq:(half + 1) * Lq] = res.results[c]["out"]
    return outp


# revision 37
# speedup vs baseline: 1.0069x; 1.0069x over previous
"""Trainium2 Bass kernel for a dense transformer encoder layer (v3).

Reference computation (per batch b):
    q = x.reshape(L, H, E)                       # H=16 heads, E=64
    scores = q @ q^T per head, scaled softmax    # A = softmax(s/8)
    new_x  = concat_h(A_h @ q_h)                 # [L, D]
    x1 = LN(x + new_x; g1, be1)
    y  = relu(x1 @ w1^T + b1) @ w2^T + b2
    out = LN(x1 + y; g2, be2)

Sharding: pure data parallel over (batch, seq-half): core c handles
batch c//2, query rows [(c%2)*1024, +1024).  Keys/values span the full
sequence of that batch, so every core gets the whole x[b] (queries
reordered first) and the full FFN weights.  No device collectives.

Structure (per core, all matmuls bf16):
  - x^T built by DMA-engine XBAR transposes; V (with a ones column per
    head for the softmax denominators) is DMA'd pre-interleaved from
    the host.
  - attention pipelined at u-tile granularity: scores(u) -> exp(u) on
    the ACT engine -> AV(u) lagging LAG u-steps so the PE never waits
    on exp.  The per-head U^T epilogue (PE transpose + divide) is
    emitted one head late as PE gap filler.
  - LN1/LN2 run in two phases over all row tiles (stats+sqrt first,
    then normalize) so the DVE->ACT->DVE latency is paid once, not per
    row tile.  be1 is folded into the FFN biases host-side.
  - FFN weights stream as per-block-contiguous [P, n*P] DMAs on the
    hardware DGE queues; each stationary serves both 512-col slabs;
    FFN2 accumulates all 32 f-tiles in PSUM.
"""

import numpy as np

import concourse.bass as bass
import concourse.tile as tile
from concourse import bacc
from concourse import mybir
from concourse.masks import make_identity

F32 = mybir.dt.float32
BF16 = mybir.dt.bfloat16
I16 = mybir.dt.int16
EXP = mybir.ActivationFunctionType.Exp

# Schraudolph-style exp in the bf16 bit domain: the bf16 bit pattern of
# exp(s/8) = 2^(s*log2(e)/8) is approximately int16(s*SCHRA_A + SCHRA_B).
# Max rel err ~3.3%; used only for off-diagonal score tiles, where no
# single softmax weight is large, so the error is ~1e-3 on the output.
SCHRA_A = float(np.log2(np.e) / 8.0 * 128.0)
SCHRA_B = 16256.0 - 5.5
RELU = mybir.ActivationFunctionType.Relu
SQRT = mybir.ActivationFunctionType.Sqrt
SUB = mybir.AluOpType.subtract
MUL = mybir.AluOpType.mult

LN_EPS = 1e-5
E = 64          # head dim
W = E + 1       # head dim + ones column
P = 128         # partitions


def build_program(S=2048, D=1024, F=4096, n_cores_unused=8):
    H = D // E
    Lq = S // 2
    ST = S // P          # s-tiles
    LT = Lq // P         # query row tiles
    DT = D // P          # d chunks
    FT = F // P          # f tiles
    SL = 512             # moving slab width
    NSL = Lq // SL
    GS = min(512, D)     # bn_stats subgroup size
    LAG = 3              # AV lags scores by LAG u-steps

    nc = bacc.Bacc("TRN2")

    xb = nc.dram_tensor("xb", [Lq, D], F32, kind="ExternalInput")
    xb16 = nc.dram_tensor("xb16", [S, D], BF16, kind="ExternalInput")
    xbv = nc.dram_tensor("xbv", [S, H, W], BF16, kind="ExternalInput")
    w1th = nc.dram_tensor("w1th", [FT, P, DT, P], BF16, kind="ExternalInput")
    w2th = nc.dram_tensor("w2th", [DT, P, FT, P], BF16, kind="ExternalInput")
    b1 = nc.dram_tensor("b1", [F], F32, kind="ExternalInput")
    b2 = nc.dram_tensor("b2", [D], F32, kind="ExternalInput")
    g1 = nc.dram_tensor("g1", [D], F32, kind="ExternalInput")
    g2 = nc.dram_tensor("g2", [D], F32, kind="ExternalInput")
    be2 = nc.dram_tensor("be2", [D], F32, kind="ExternalInput")
    out = nc.dram_tensor("out", [Lq, D], F32, kind="ExternalOutput")

    def bcast(dram_vec, n):
        a = dram_vec[:]
        return bass.AP(tensor=a.tensor, offset=a.offset, ap=[[0, P]] + a.ap)

    with tile.TileContext(nc) as tc:
        with (
            tc.tile_pool(name="persist", bufs=1) as persist,
            tc.tile_pool(name="small", bufs=6) as small,
            tc.tile_pool(name="w1p", bufs=4) as w1p,
        ):
            ident = persist.tile([P, P], F32)
            make_identity(nc, ident)
            ident16 = persist.tile([P, P], BF16)
            make_identity(nc, ident16)
            b1s = persist.tile([P, FT], F32)
            nc.sync.dma_start(out=b1s, in_=b1[:].rearrange("(t p) -> p t", p=P))
            b2s = persist.tile([P, DT], F32)
            nc.sync.dma_start(out=b2s, in_=b2[:].rearrange("(t p) -> p t", p=P))
            epst = persist.tile([P, 1], F32)
            nc.vector.memset(epst, LN_EPS)
            g1b = persist.tile([P, D], F32)
            nc.gpsimd.dma_start(out=g1b, in_=bcast(g1, D))
            g2b = persist.tile([P, D], F32)
            nc.gpsimd.dma_start(out=g2b, in_=bcast(g2, D))
            be2b = persist.tile([P, D], F32)
            nc.gpsimd.dma_start(out=be2b, in_=bcast(be2, D))
            # new_x: attention output, then r1/LN1 scratch, then r2.
            new_x = persist.tile([P, LT, D], F32)
            x1b = persist.tile([P, LT, D], BF16)   # LN1 out * g1 (no be1)
            x1T = persist.tile([P, DT, Lq], BF16)  # x1b transposed
            mvall = persist.tile([P, LT, 2], F32)  # bn mean/var per row tile
            rsall = persist.tile([P, LT, 1], F32)  # rstd per row tile

            # ---------------- attention ----------------
            with (
                tc.tile_pool(name="asb", bufs=1) as asb,
                tc.tile_pool(name="etp", bufs=5) as etp,
                tc.tile_pool(name="utsp", bufs=2) as utsp,
                tc.tile_pool(name="xqp", bufs=8) as xqp,
                tc.tile_pool(name="scp", bufs=2, space="PSUM") as scp,
                tc.tile_pool(name="utp", bufs=2, space="PSUM") as utp,
                tc.tile_pool(name="upp", bufs=2, space="PSUM") as upp,
            ):
                vaug = asb.tile([P, ST, H, W], BF16)
                xT = asb.tile([P, DT, S], BF16)

                def emit_xt(t):
                    nc.sync.dma_start_transpose(
                        out=xT[:, t, :], in_=xb16[:, t * P:(t + 1) * P])

                def emit_vaug(u):
                    nc.sync.dma_start(
                        out=vaug[:, u, :, :],
                        in_=xbv[u * P:(u + 1) * P, :, :])

                # head 0 needs xT col 0 + the first vaug tiles first.
                emit_xt(0)
                for u in range(4):
                    emit_vaug(u)
                emit_xt(1)
                for u in range(4, ST):
                    emit_vaug(u)
                for t in range(2, DT):
                    emit_xt(t)

                uts_saved = {}

                def emit_epilogue(h):
                    # transpose U^T [65, Lq] back to [l, 65] in 128-row
                    # tiles, divide by the rowsum riding in column 64.
                    uts = uts_saved.pop(h)
                    for half in range(LT // 4):
                        up = upp.tile([P, 4, W], F32, tag="up")
                        for j in range(4):
                            lt = half * 4 + j
                            nc.tensor.transpose(
                                up[:, j, :], uts[:, lt * P:(lt + 1) * P],
                                ident[0:W, 0:W])
                        nc.vector.reciprocal(
                            out=up[:, :, E:W], in_=up[:, :, E:W])
                        for j in range(4):
                            lt = half * 4 + j
                            nc.vector.tensor_scalar_mul(
                                out=new_x[:, lt, h * E:(h + 1) * E],
                                in0=up[:, j, 0:E], scalar1=up[:, j, E:W])

                for h in range(H):
                    t, ro = h // 2, (h % 2) * E
                    ut_sl = [utp.tile([W, SL], F32, tag="ut", name="ut")
                             for _ in range(NSL)]
                    ets = {}

                    def emit_scores(u):
                        sc = scp.tile([P, Lq], F32, tag="sc")
                        for sl in range(NSL):
                            nc.tensor.matmul(
                                sc[:, sl * SL:(sl + 1) * SL],
                                xT[ro:ro + E, t, u * P:(u + 1) * P],
                                xT[ro:ro + E, t, sl * SL:(sl + 1) * SL],
                                start=True, stop=True)
                        et = etp.tile([P, Lq], BF16, tag="et")
                        nc.scalar.activation(
                            out=et, in_=sc, func=EXP, scale=1.0 / 8.0)
                        ets[u] = et

                    def emit_av(u):
                        et = ets.pop(u)
                        for sl in range(NSL):
                            nc.tensor.matmul(
                                ut_sl[sl], vaug[:, u, h, 0:W],
                                et[:, sl * SL:(sl + 1) * SL],
                                start=(u == 0), stop=(u == ST - 1))

                    for u in range(ST):
                        emit_scores(u)
                        if u == 2 and h > 0:
                            emit_epilogue(h - 1)
                        if u >= LAG:
                            emit_av(u - LAG)
                    for u in range(ST - LAG, ST):
                        emit_av(u)

                    uts = utsp.tile([W, Lq], F32, tag="uts")
                    for sl in range(NSL):
                        nc.vector.tensor_copy(
                            out=uts[:, sl * SL:(sl + 1) * SL], in_=ut_sl[sl])
                    uts_saved[h] = uts
                emit_epilogue(H - 1)

                # residual 1 + LN1 (be1 folded into the FFN biases).
                # Two-phase over row tiles so the DVE->ACT->DVE hop for
                # rsqrt is paid in parallel across tiles.
                for lt in range(LT):
                    xq = xqp.tile([P, D], F32, tag="xq")
                    nc.sync.dma_start(
                        out=xq, in_=xb[lt * P:(lt + 1) * P, :])
                    nc.gpsimd.tensor_add(
                        out=new_x[:, lt, :], in0=new_x[:, lt, :], in1=xq)
                    _ln_stats(nc, small, new_x[:, lt, :],
                              mvall[:, lt, :], rsall[:, lt, :], epst, GS)
                for lt in range(LT):
                    nc.vector.reciprocal(
                        out=rsall[:, lt, :], in_=rsall[:, lt, :])
                    nc.vector.tensor_scalar(
                        out=new_x[:, lt, :], in0=new_x[:, lt, :],
                        scalar1=mvall[:, lt, 0:1], scalar2=rsall[:, lt, :],
                        op0=SUB, op1=MUL)
                    nc.vector.tensor_mul(
                        out=x1b[:, lt, :], in0=new_x[:, lt, :], in1=g1b)

            # ---------------- x1 transpose + FFN (fp8 DoubleRow) -------
            with (
                tc.tile_pool(name="fsb", bufs=1) as fsb,
                tc.tile_pool(name="w2p", bufs=2) as w2p,
                tc.tile_pool(name="ybp", bufs=2) as ybp,
                tc.tile_pool(name="outp", bufs=3) as outp,
            ):
                with tc.tile_pool(name="x1tp", bufs=4, space="PSUM") as x1tp:
                    for lt in range(LT):
                        for dc in range(DT):
                            tp = x1tp.tile([P, P], BF16)
                            nc.tensor.transpose(
                                tp, x1b[:, lt, dc * P:(dc + 1) * P], ident16)
                            nc.vector.tensor_copy(
                                out=x1T[:, dc, lt * P:(lt + 1) * P], in_=tp)

                htall = fsb.tile([P, FT, Lq], BF16)

                with tc.tile_pool(name="hpp", bufs=4, space="PSUM") as hpp:
                    for ft in range(FT):
                        wblk = w1p.tile([P, DT, P], BF16, tag="w1")
                        nc.sync.dma_start(out=wblk, in_=w1th[ft, :, :, :])
                        hp = [hpp.tile([P, SL], F32, tag="hp", name="hp")
                              for _ in range(NSL)]
                        for dc in range(DT):
                            stat = wblk[:, dc, :]
                            for sl in range(NSL):
                                nc.tensor.matmul(
                                    hp[sl], stat,
                                    x1T[:, dc, sl * SL:(sl + 1) * SL],
                                    start=(dc == 0), stop=(dc == DT - 1))
                        for sl in range(NSL):
                            nc.scalar.activation(
                                out=htall[:, ft, sl * SL:(sl + 1) * SL],
                                in_=hp[sl], func=RELU, bias=b1s[:, ft:ft + 1])

                with (
                    tc.tile_pool(name="ypp", bufs=4, space="PSUM") as ypp,
                    tc.tile_pool(name="ytp", bufs=2, space="PSUM") as ytp,
                ):
                    for dt in range(DT):
                        w2blk = w2p.tile([P, FT, P], BF16, tag="w2")
                        nc.scalar.dma_start(out=w2blk, in_=w2th[dt, :, :, :])
                        yp = [ypp.tile([P, SL], F32, tag="yp", name="yp")
                              for _ in range(NSL)]
                        for j in range(FT):
                            stat = w2blk[:, j, :]
                            for sl in range(NSL):
                                nc.tensor.matmul(
                                    yp[sl], stat,
                                    htall[:, j, sl * SL:(sl + 1) * SL],
                                    start=(j == 0), stop=(j == FT - 1))
                        for sl in range(NSL):
                            ybuf = ybp.tile([P, SL], F32, tag="yb")
                            nc.vector.tensor_scalar_add(
                                out=ybuf, in0=yp[sl],
                                scalar1=b2s[:, dt:dt + 1])
                            for c in range(SL // P):
                                lt = sl * (SL // P) + c
                                yt = ytp.tile([P, P], F32)
                                nc.tensor.transpose(
                                    yt, ybuf[:, c * P:(c + 1) * P], ident)
                                nc.vector.tensor_add(
                                    out=new_x[:, lt, dt * P:(dt + 1) * P],
                                    in0=x1b[:, lt, dt * P:(dt + 1) * P],
                                    in1=yt)

                    # LN2 -> out (two-phase again)
                    for lt in range(LT):
                        _ln_stats(nc, small, new_x[:, lt, :],
                                  mvall[:, lt, :], rsall[:, lt, :], epst, GS)
                    for lt in range(LT):
                        nc.vector.reciprocal(
                            out=rsall[:, lt, :], in_=rsall[:, lt, :])
                        nc.vector.tensor_scalar(
                            out=new_x[:, lt, :], in0=new_x[:, lt, :],
                            scalar1=mvall[:, lt, 0:1], scalar2=rsall[:, lt, :],
                            op0=SUB, op1=MUL)
                        ot = outp.tile([P, D], F32, tag="ot")
                        nc.vector.tensor_mul(
                            out=ot, in0=new_x[:, lt, :], in1=g2b)
                        nc.vector.tensor_add(out=ot, in0=ot, in1=be2b)
                        nc.sync.dma_start(
                            out=out[lt * P:(lt + 1) * P, :], in_=ot)

    nc.finalize()
    return nc


def _ln_stats(nc, small, x_ap, mv, rstd, epst, GS):
    """mv <- [mean, var] of x over free dim; rstd <- sqrt(var + eps)."""
    D = x_ap.shape[-1]
    ngr = D // GS
    st = small.tile([P, ngr, 6], F32, tag="bnst")
    xg = x_ap.rearrange("p (g k) -> p g k", k=GS)
    for g in range(ngr):
        nc.vector.bn_stats(out=st[:, g, :], in_=xg[:, g, :])
    nc.vector.bn_aggr(out=mv, in_=st)
    nc.scalar.activation(out=rstd, in_=mv[:, 1:2], func=SQRT, bias=epst)


# ---------------------------------------------------------------------------
# host side
# ---------------------------------------------------------------------------

_PROG_CACHE = {}


def get_program(S=2048, D=1024, F=4096):
    key = (S, D, F)
    if key not in _PROG_CACHE:
        _PROG_CACHE[key] = build_program(S, D, F)
    return _PROG_CACHE[key]


def make_in_maps(x, w1, b1, w2, b2, g1, be1, g2, be2, n_cores=8):
    B, L, D = x.shape
    F = w1.shape[0]
    Lq = L // 2
    H, Wd = D // 64, 65
    DT, FT = D // 128, F // 128
    import ml_dtypes
    # per-block-contiguous weight layouts:
    # w1th[ft, p, dc, q] = w1[ft*128+q, dc*128+p]
    w1th = np.ascontiguousarray(
        w1.T.reshape(DT, 128, FT, 128).transpose(2, 1, 0, 3)).astype(ml_dtypes.bfloat16)
    # w2th[dt, p, j, q] = w2[dt*128+q, j*128+p]
    w2th = np.ascontiguousarray(
        w2.T.reshape(FT, 128, DT, 128).transpose(2, 1, 0, 3)).astype(ml_dtypes.bfloat16)
    # be1 folded into the FFN biases: relu(w1 @ (x1 + be1) + b1) =
    # relu(w1 @ x1 + b1'), and r2 = x1 + y + be1 = x1 + (w2 h + b2').
    b1f = (b1 + w1 @ be1).astype(np.float32)
    b2f = (b2 + be1).astype(np.float32)
    common = dict(w1th=w1th, w2th=w2th, b1=b1f, b2=b2f, g1=g1, g2=g2, be2=be2)
    in_maps = []
    for c in range(n_cores):
        b, half = c // 2, c % 2
        lo = half * Lq
        xq = x[b, lo:lo + Lq]
        xo = x[b, Lq - lo:2 * Lq - lo]
        xbl = np.ascontiguousarray(np.concatenate([xq, xo], axis=0))
        # V with a ones column interleaved per head: [S, H, 65]
        xv = np.concatenate(
            [xbl.reshape(L, H, 64), np.ones((L, H, 1), np.float32)],
            axis=2)
        in_maps.append(dict(xb=np.ascontiguousarray(xq),
                            xb16=xbl.astype(ml_dtypes.bfloat16),
                            xbv=np.ascontiguousarray(xv).astype(ml_dtypes.bfloat16),
                            **common))
    return in_maps


def kernel(x, w1, b1, w2, b2, g1, be1, g2, be2):
    from concourse.bass_utils import run_bass_kernel_spmd

    x = np.asarray(x, dtype=np.float32)
    B, L, D = x.shape
    F = w1.shape[0]
    Lq = L // 2
    n_cores = 2 * B
    nc = get_program(L, D, F)
    in_maps = make_in_maps(x, np.asarray(w1, np.float32), np.asarray(b1, np.float32),
                           np.asarray(w2, np.float32), np.asarray(b2, np.float32),
                           np.asarray(g1, np.float32), np.asarray(be1, np.float32),
                           np.asarray(g2, np.float32), np.asarray(be2, np.float32),
                           n_cores)
    res = run_bass_kernel_spmd(nc, in_maps, core_ids=list(range(n_cores)))
    outp = np.empty((B, L, D), dtype=np.float32)
    for c in range(n_cores):
        b, half = c // 2, c % 2
        outp[b, half * Lq:(half + 1) * Lq] = res.results[c]["out"]
    return outp
